# revision 19
# baseline (speedup 1.0000x reference)
"""ChebyKAN layer on 8 Trainium2 NeuronCores (data-parallel over batch).

Computation:  out[b,o] = sum_{i,d} T_d(tanh(x)[b,i]) * C[i,o,d]
  - batch 32768 sharded 8 ways (4096 rows/core), coefficients replicated.
  - Per core: x-shard pre-transposed on host to [i=512, b=4096] so Chebyshev
    tiles sit [i partitions, b free]; PE contracts over (i,d) with cheby tiles
    as the stationary operand and C chunks [i,o] as the moving operand,
    accumulating out[b_tile=128, o=512] in PSUM over 33 chunks of 128.
  - d=0 (T_0 == 1) is folded: its four i-chunks are pre-summed on host into a
    single [128,512] chunk matmul'd against a constant ones tile.
"""

import os
from functools import lru_cache

import numpy as np
import ml_dtypes

import concourse.bass as bass
import concourse.mybir as mybir
import concourse.tile as tile
from concourse import bacc
from concourse.bass_utils import run_bass_kernel_spmd

N_CORES = 8
BATCH, IN_F, OUT_F, DEG = 32768, 512, 512, 8
B_LOC = BATCH // N_CORES  # 4096
P = 128
N_ICHUNK = IN_F // P  # 4
N_KCHUNK = 1 + DEG * N_ICHUNK  # 33 (1 folded d=0 chunk + 32)

MM_DT_NAME = os.environ.get("CHEBY_MM_DT", "bf16")
_DT = {
    "bf16": (mybir.dt.bfloat16, ml_dtypes.bfloat16),
    "f32": (mybir.dt.float32, np.float32),
    "f32r": (mybir.dt.float32r, np.float32),
}
MM_DT, MM_NP = _DT[MM_DT_NAME]
# block of batch columns processed per iteration (SBUF-resident cheby tiles)
BBLK = 512 if MM_DT_NAME == "bf16" else 256


def _build_kernel(reps=1):
    f32 = mybir.dt.float32
    nc = bacc.Bacc(
        "TRN2",
        target_bir_lowering=False,
        debug=False,
        num_devices=N_CORES,
    )
    xT = nc.declare_dram_parameter("xT", [IN_F, B_LOC], f32, isOutput=False)
    cw = nc.declare_dram_parameter("Cw", [N_KCHUNK * P, OUT_F], MM_DT, isOutput=False)
    out = nc.declare_dram_parameter("out", [B_LOC, OUT_F], f32, isOutput=True)

    xT_ap = xT[:, :].rearrange("(c p) b -> p c b", p=P)  # [128, 4, B_LOC]
    cw_ap = cw[:, :].rearrange("(k p) o -> p k o", p=P)  # [128, 33, 512]

    import contextlib

    with tile.TileContext(nc) as tc:
        with (
            tc.tile_pool(name="const", bufs=1) as const_pool,
            tc.tile_pool(name="xin", bufs=3) as xin_pool,
            tc.tile_pool(name="tf32", bufs=1) as f32_pool,
            tc.tile_pool(name="cheb", bufs=2) as cheb_pool,
            tc.tile_pool(name="ot", bufs=4) as out_pool,
            tc.tile_pool(name="ps", bufs=3, space="PSUM") as psum_pool,
        ):
            c_tile = const_pool.tile([P, N_KCHUNK, OUT_F], MM_DT)
            # split the C load so early k-chunks land before the first matmuls
            nsplit = 4
            per = (N_KCHUNK + nsplit - 1) // nsplit
            for s in range(nsplit):
                k0, k1 = s * per, min((s + 1) * per, N_KCHUNK)
                nc.gpsimd.dma_start(
                    out=c_tile[:, k0:k1, :], in_=cw_ap[:, k0:k1, :]
                )
            ones = const_pool.tile([P, P], MM_DT)
            nc.vector.memset(ones[:, :], 1.0)

            rep_ctx = tc.For_i(0, reps, 1) if reps > 1 else contextlib.nullcontext()
            with rep_ctx:
                _kernel_body(nc, tc, xT_ap, c_tile, ones, out,
                             xin_pool, f32_pool, cheb_pool, out_pool, psum_pool)
    nc.compile()
    return nc


def _kernel_body(nc, tc, xT_ap, c_tile, ones, out,
                 xin_pool, f32_pool, cheb_pool, out_pool, psum_pool):
    f32 = mybir.dt.float32
    MULT = mybir.AluOpType.mult
    ACT_F = mybir.ActivationFunctionType

    def stt(o, a, b):  # o = 2*a*b
        nc.vector.scalar_tensor_tensor(
            out=o, in0=a, scalar=2.0, in1=b, op0=MULT, op1=MULT
        )

    def sub1(o):  # o -= 1
        nc.vector.tensor_scalar(
            out=o, in0=o, scalar1=1.0, scalar2=None,
            op0=mybir.AluOpType.subtract,
        )

    for blk in range(B_LOC // BBLK):
        b0 = blk * BBLK
        x_in = xin_pool.tile([P, N_ICHUNK, BBLK], f32)
        nc.sync.dma_start(out=x_in[:, :, :], in_=xT_ap[:, :, b0 : b0 + BBLK])

        # Tf[:, j] = T_{j+1} in fp32 (j=0..3); Tb[:, j] = T_{j+1} in bf16 (j=0..7)
        Tf = f32_pool.tile([P, 4, N_ICHUNK, BBLK], f32)
        Tb = cheb_pool.tile([P, DEG, N_ICHUNK, BBLK], MM_DT)
        t1, t2, t3, t4 = (Tf[:, j, :, :] for j in range(4))
        nc.scalar.activation(out=t1, in_=x_in[:, :, :], func=ACT_F.Tanh)
        # fp32 chain: T2=2T1^2-1, T3=2T2T1-T1, T4=2T2^2-1
        stt(t2, t1, t1); sub1(t2)
        stt(t3, t2, t1); nc.vector.tensor_sub(t3, t3, t1)
        stt(t4, t2, t2); sub1(t4)
        # one-time rounding to bf16 on the scalar engine
        for j in range(4):
            nc.scalar.copy(out=Tb[:, j, :, :], in_=Tf[:, j, :, :])
        b1, b2, b3, b4 = (Tb[:, j, :, :] for j in range(4))
        b5, b6, b7, b8 = (Tb[:, j, :, :] for j in range(4, 8))
        # bf16 products: T5=2T3T2-T1, T6=2T3^2-1, T7=2T4T3-T1, T8=2T4^2-1
        stt(b5, b3, b2); nc.vector.tensor_sub(b5, b5, b1)
        stt(b6, b3, b3); sub1(b6)
        stt(b7, b4, b3); nc.vector.tensor_sub(b7, b7, b1)
        stt(b8, b4, b4); sub1(b8)

        for bt in range(BBLK // P):
            # Split each K=128 matmul into two concurrent K=64 row-group
            # tiles (base partitions 0 and 64). Disjoint row groups execute
            # simultaneously on the PE array and each half's LDWEIGHTS
            # overlaps the other half's streaming, hiding the ~53ns weight
            # load that is otherwise exposed per matmul. The halves
            # accumulate in separate PSUM banks, combined at copy-out.
            H = P // 2
            ps = [
                psum_pool.tile(
                    [P, OUT_F], f32, space="PSUM", tag=f"ps{h}", name=f"ps{h}"
                )
                for h in range(2)
            ]
            bsl = slice(bt * P, (bt + 1) * P)
            for h, lo in ((0, 0), (1, H)):
                nc.tensor.matmul(
                    ps[h][:, :],
                    ones[lo : lo + H, :],
                    c_tile[lo : lo + H, 0, :],
                    start=True,
                    stop=False,
                )
            for j in range(DEG):
                for c in range(N_ICHUNK):
                    k = 1 + j * N_ICHUNK + c
                    stop = k == N_KCHUNK - 1
                    for h, lo in ((0, 0), (1, H)):
                        nc.tensor.matmul(
                            ps[h][:, :],
                            Tb[lo : lo + H, j, c, bsl],
                            c_tile[lo : lo + H, k, :],
                            start=False,
                            stop=stop,
                        )
            o_tile = out_pool.tile([P, OUT_F], f32)
            half_sb = out_pool.tile([P, OUT_F], f32, tag="halfsb")
            nc.scalar.copy(out=half_sb[:, :], in_=ps[0][:, :])
            nc.vector.tensor_add(o_tile[:, :], half_sb[:, :], ps[1][:, :])
            row = b0 + bt * P
            nc.sync.dma_start(out=out[row : row + P, :], in_=o_tile[:, :])


@lru_cache(maxsize=4)
def _get_nc(reps=1):
    return _build_kernel(reps)


class Runner:
    """Persistent jitted runner mirroring bass2jax.run_bass_via_pjrt, reusable
    across calls (single jit cache entry) so repeated executions can be timed
    back-to-back without recompilation or host round-trips per call."""

    def __init__(self, nc):
        import jax
        import jax.numpy as jnp
        from jax.sharding import Mesh, PartitionSpec
        from jax.experimental.shard_map import shard_map
        from concourse import bass2jax
        from concourse import mybir as _mybir

        bass2jax.install_neuronx_cc_hook()
        self.jax = jax
        self.nc = nc
        partition_name = (
            nc.partition_id_tensor.name if nc.partition_id_tensor else None
        )
        in_names, out_names, out_avals = [], [], []
        for alloc in nc.m.functions[0].allocations:
            if not isinstance(alloc, _mybir.MemoryLocationSet):
                continue
            name = alloc.memorylocations[0].name
            if alloc.kind == "ExternalInput":
                if name != partition_name:
                    in_names.append(name)
            elif alloc.kind == "ExternalOutput":
                out_names.append(name)
                out_avals.append(
                    jax.core.ShapedArray(
                        tuple(alloc.tensor_shape), _mybir.dt.np(alloc.dtype)
                    )
                )
        self.in_names = list(in_names)
        self.out_names = out_names
        self.out_avals = out_avals
        n_params = len(in_names)
        all_names = in_names + out_names
        if partition_name is not None:
            all_names = all_names + [partition_name]

        def _body(*args):
            operands = list(args)
            if partition_name is not None:
                operands.append(bass2jax.partition_id_tensor())
            return tuple(
                bass2jax._bass_exec_p.bind(
                    *operands,
                    out_avals=tuple(out_avals),
                    in_names=tuple(all_names),
                    out_names=tuple(out_names),
                    lowering_input_output_aliases=(),
                    sim_require_finite=True,
                    sim_require_nnan=True,
                    nc=nc,
                )
            )

        devices = jax.devices()[:N_CORES]
        self.mesh = Mesh(np.asarray(devices), ("core",))
        in_specs = (PartitionSpec("core"),) * (n_params + len(out_names))
        out_specs = (PartitionSpec("core"),) * len(out_names)
        self.fn = jax.jit(
            shard_map(
                _body,
                mesh=self.mesh,
                in_specs=in_specs,
                out_specs=out_specs,
                check_rep=False,
            ),
            keep_unused=True,
        )

    def put_inputs(self, in_maps):
        import jax
        from jax.sharding import NamedSharding, PartitionSpec

        concat = [
            np.concatenate([np.asarray(m[name]) for m in in_maps], axis=0)
            for name in self.in_names
        ]
        for aval in self.out_avals:
            concat.append(
                np.zeros((N_CORES * aval.shape[0], *aval.shape[1:]), aval.dtype)
            )
        sh = NamedSharding(self.mesh, PartitionSpec("core"))
        return [jax.device_put(a, sh) for a in concat]

    def __call__(self, dev_inputs):
        return self.fn(*dev_inputs)

    def run_np(self, in_maps):
        outs = self(self.put_inputs(in_maps))
        return [
            {
                name: np.asarray(outs[i]).reshape(N_CORES, *self.out_avals[i].shape)[c]
                for i, name in enumerate(self.out_names)
            }
            for c in range(N_CORES)
        ]


def _prep_inputs(x: np.ndarray, coefficients: np.ndarray):
    x = np.asarray(x, dtype=np.float32)
    coefficients = np.asarray(coefficients, dtype=np.float32)
    # C chunks: k=0 is the d=0 term pre-summed over its 4 i-chunks (T_0 == 1);
    # k=1+j*4+c is degree j+1, i-chunk c, laid out [i within chunk, o].
    c_perm = np.transpose(coefficients, (2, 0, 1))  # (d, i, o)
    c0 = c_perm[0].reshape(N_ICHUNK, P, OUT_F).sum(axis=0)  # (128, 512)
    c_main = c_perm[1:].reshape(DEG * N_ICHUNK, P, OUT_F)
    c_all = np.concatenate([c0[None], c_main], axis=0)
    c_all = np.ascontiguousarray(c_all.reshape(N_KCHUNK * P, OUT_F)).astype(MM_NP)

    in_maps = []
    for core in range(N_CORES):
        shard = x[core * B_LOC : (core + 1) * B_LOC]  # (4096, 512)
        xt = np.ascontiguousarray(shard.T)  # (512, 4096)
        in_maps.append({"xT": xt, "Cw": c_all})
    return in_maps


@lru_cache(maxsize=4)
def _get_runner(reps=1):
    return Runner(_get_nc(reps))


def run_sharded(x, coefficients):
    """Run the 8-core kernel; returns the full (32768, 512) float32 output."""
    in_maps = _prep_inputs(x, coefficients)
    runner = _get_runner()
    results = runner.run_np(in_maps)
    parts = [np.asarray(results[i]["out"]) for i in range(N_CORES)]
    return np.concatenate(parts, axis=0).astype(np.float32)


def _time_runner(runner, dev_in, iters):
    import time

    outs = runner(dev_in)  # warm up
    outs[0].block_until_ready()
    times = []
    for _ in range(iters):
        t0 = time.perf_counter()
        outs = runner(dev_in)
        outs[0].block_until_ready()
        times.append((time.perf_counter() - t0) * 1e9)
    return times


def bench(x, coefficients, iters=12, rep_a=3, rep_b=83):
    """Estimate per-invocation HW time from the slope between two on-device
    repeat counts (fixed ~66-107ms axon RPC overhead cancels). Interleaved
    rounds + median to reject the bimodal RPC jitter. Returns
    (slope_ns, times_a, times_b)."""
    in_maps = _prep_inputs(x, coefficients)
    ra, rb = _get_runner(rep_a), _get_runner(rep_b)
    dev_a = ra.put_inputs(in_maps)
    dev_b = rb.put_inputs(in_maps)
    ta, tb = [], []
    for _ in range(3):
        ta += _time_runner(ra, dev_a, iters // 3 + 1)
        tb += _time_runner(rb, dev_b, iters // 3 + 1)
    med = lambda t: sorted(t)[len(t) // 2]
    slope = (med(tb) - med(ta)) / (rep_b - rep_a)
    return slope, ta, tb


def kernel(x, coefficients):
    return run_sharded(x, coefficients)


# revision 27
# speedup vs baseline: 1.0946x; 1.0946x over previous
"""ChebyKAN layer on 8 Trainium2 NeuronCores (data-parallel over batch).

Computation:  out[b,o] = sum_{i,d} T_d(tanh(x)[b,i]) * C[i,o,d]
  - batch 32768 sharded 8 ways (4096 rows/core), coefficients replicated.
  - Per core: x-shard pre-transposed on host to [i=512, b=4096] so Chebyshev
    tiles sit [i partitions, b free]; PE contracts over (i,d) with cheby tiles
    as the stationary operand and C chunks [i,o] as the moving operand,
    accumulating out[b_tile=128, o=512] in PSUM over 33 chunks of 128.
  - d=0 (T_0 == 1) is folded: its four i-chunks are pre-summed on host into a
    single [128,512] chunk matmul'd against a constant ones tile.
"""

import os
from functools import lru_cache

import numpy as np
import ml_dtypes

import concourse.bass as bass
import concourse.mybir as mybir
import concourse.tile as tile
from concourse import bacc
from concourse.bass_utils import run_bass_kernel_spmd

N_CORES = 8
BATCH, IN_F, OUT_F, DEG = 32768, 512, 512, 8
B_LOC = BATCH // N_CORES  # 4096
P = 128
N_ICHUNK = IN_F // P  # 4
N_KCHUNK = DEG * N_ICHUNK  # 32 (d=0 handled as a bias add at copy-out)

MM_DT_NAME = os.environ.get("CHEBY_MM_DT", "f16")
_DT = {
    "bf16": (mybir.dt.bfloat16, ml_dtypes.bfloat16),
    "f16": (mybir.dt.float16, np.float16),
    "f32": (mybir.dt.float32, np.float32),
    "f32r": (mybir.dt.float32r, np.float32),
}
MM_DT, MM_NP = _DT[MM_DT_NAME]
# block of batch columns processed per iteration (SBUF-resident cheby tiles)
BBLK = 512 if MM_DT_NAME in ("bf16", "f16") else 256
# 1 = single K=128 matmul per chunk; 2 = two concurrent K=64 row-group tiles
KSPLIT = int(os.environ.get("CHEBY_KSPLIT", "1"))
# coefficients scaled up on host so fp16 C stays normal; undone at copy-out
C_SCALE = 1024.0 if MM_DT_NAME == "f16" else 1.0


def _build_kernel(reps=1):
    f32 = mybir.dt.float32
    nc = bacc.Bacc(
        "TRN2",
        target_bir_lowering=False,
        debug=False,
        num_devices=N_CORES,
    )
    xT = nc.declare_dram_parameter("xT", [IN_F, B_LOC], f32, isOutput=False)
    cw = nc.declare_dram_parameter("Cw", [N_KCHUNK * P, OUT_F], MM_DT, isOutput=False)
    bias = nc.declare_dram_parameter("bias", [1, OUT_F], f32, isOutput=False)
    out = nc.declare_dram_parameter("out", [B_LOC, OUT_F], f32, isOutput=True)

    xT_ap = xT[:, :].rearrange("(c p) b -> p c b", p=P)  # [128, 4, B_LOC]
    cw_ap = cw[:, :].rearrange("(k p) o -> p k o", p=P)  # [128, 32, 512]

    import contextlib

    with tile.TileContext(nc) as tc:
        with (
            tc.tile_pool(name="const", bufs=1) as const_pool,
            tc.tile_pool(name="xin", bufs=3) as xin_pool,
            tc.tile_pool(name="tf32", bufs=1) as f32_pool,
            tc.tile_pool(name="cheb", bufs=2) as cheb_pool,
            tc.tile_pool(name="ot", bufs=4) as out_pool,
            tc.tile_pool(name="ps", bufs=6 // KSPLIT, space="PSUM") as psum_pool,
        ):
            c_tile = const_pool.tile([P, N_KCHUNK, OUT_F], MM_DT)
            # split the C load so early k-chunks land before the first matmuls
            nsplit = 4
            per = (N_KCHUNK + nsplit - 1) // nsplit
            for s in range(nsplit):
                k0, k1 = s * per, min((s + 1) * per, N_KCHUNK)
                nc.gpsimd.dma_start(
                    out=c_tile[:, k0:k1, :], in_=cw_ap[:, k0:k1, :]
                )
            # bias row (the folded d=0 term) broadcast to all 128 partitions
            b_tile = const_pool.tile([P, OUT_F], f32)
            bias_ap = bias[:, :]
            bias_bcast = bass.AP(
                tensor=bias_ap.tensor,
                offset=bias_ap.offset,
                ap=[[0, P], bias_ap.ap[1]],
            )
            nc.gpsimd.dma_start(out=b_tile[:, :], in_=bias_bcast)

            rep_ctx = tc.For_i(0, reps, 1) if reps > 1 else contextlib.nullcontext()
            with rep_ctx:
                _kernel_body(nc, tc, xT_ap, c_tile, b_tile, out,
                             xin_pool, f32_pool, cheb_pool, out_pool, psum_pool)
    nc.compile()
    return nc


def _kernel_body(nc, tc, xT_ap, c_tile, b_tile, out,
                 xin_pool, f32_pool, cheb_pool, out_pool, psum_pool):
    f32 = mybir.dt.float32
    MULT = mybir.AluOpType.mult
    ACT_F = mybir.ActivationFunctionType

    def stt(o, a, b):  # o = 2*a*b
        nc.vector.scalar_tensor_tensor(
            out=o, in0=a, scalar=2.0, in1=b, op0=MULT, op1=MULT
        )

    def sub1(o):  # o -= 1
        nc.vector.tensor_scalar(
            out=o, in0=o, scalar1=1.0, scalar2=None,
            op0=mybir.AluOpType.subtract,
        )

    for blk in range(B_LOC // BBLK):
        b0 = blk * BBLK
        x_in = xin_pool.tile([P, N_ICHUNK, BBLK], f32)
        nc.sync.dma_start(out=x_in[:, :, :], in_=xT_ap[:, :, b0 : b0 + BBLK])

        # Tf[:, j] = T_{j+1} in fp32 (j=0..3); Tb[:, j] = T_{j+1} in bf16 (j=0..7)
        Tf = f32_pool.tile([P, 4, N_ICHUNK, BBLK], f32)
        Tb = cheb_pool.tile([P, DEG, N_ICHUNK, BBLK], MM_DT)
        t1, t2, t3, t4 = (Tf[:, j, :, :] for j in range(4))
        nc.scalar.activation(out=t1, in_=x_in[:, :, :], func=ACT_F.Tanh)
        # fp32 chain: T2=2T1^2-1, T3=2T2T1-T1, T4=2T2^2-1
        stt(t2, t1, t1); sub1(t2)
        stt(t3, t2, t1); nc.vector.tensor_sub(t3, t3, t1)
        stt(t4, t2, t2); sub1(t4)
        # one-time rounding to bf16 on the scalar engine
        for j in range(4):
            nc.scalar.copy(out=Tb[:, j, :, :], in_=Tf[:, j, :, :])
        b1, b2, b3, b4 = (Tb[:, j, :, :] for j in range(4))
        b5, b6, b7, b8 = (Tb[:, j, :, :] for j in range(4, 8))
        # bf16 products: T5=2T3T2-T1, T6=2T3^2-1, T7=2T4T3-T1, T8=2T4^2-1
        stt(b5, b3, b2); nc.vector.tensor_sub(b5, b5, b1)
        stt(b6, b3, b3); sub1(b6)
        stt(b7, b4, b3); nc.vector.tensor_sub(b7, b7, b1)
        stt(b8, b4, b4); sub1(b8)

        for bt in range(BBLK // P):
            H = P // KSPLIT
            halves = [(h, h * H) for h in range(KSPLIT)]
            ps = [
                psum_pool.tile(
                    [P, OUT_F], f32, space="PSUM", tag=f"ps{h}", name=f"ps{h}"
                )
                for h in range(KSPLIT)
            ]
            bsl = slice(bt * P, (bt + 1) * P)
            for j in range(DEG):
                for c in range(N_ICHUNK):
                    k = j * N_ICHUNK + c
                    for h, lo in halves:
                        nc.tensor.matmul(
                            ps[h][:, :],
                            Tb[lo : lo + H, j, c, bsl],
                            c_tile[lo : lo + H, k, :],
                            start=(k == 0),
                            stop=(k == N_KCHUNK - 1),
                        )
            o_tile = out_pool.tile([P, OUT_F], f32)
            row = b0 + bt * P
            acc = ps[0][:, :]
            if KSPLIT > 1:
                half_sb = out_pool.tile([P, OUT_F], f32, tag="halfsb")
                nc.scalar.copy(out=half_sb[:, :], in_=ps[0][:, :])
                for h in range(1, KSPLIT - 1):
                    nc.vector.tensor_add(half_sb[:, :], half_sb[:, :], ps[h][:, :])
                nc.vector.tensor_add(half_sb[:, :], half_sb[:, :], ps[KSPLIT - 1][:, :])
                acc = half_sb[:, :]
            # out = psum / C_SCALE + bias   (bias = sum_i C[i,:,0], the d=0 term)
            nc.vector.scalar_tensor_tensor(
                out=o_tile[:, :],
                in0=acc,
                scalar=1.0 / C_SCALE,
                in1=b_tile[:, :],
                op0=MULT,
                op1=mybir.AluOpType.add,
            )
            nc.sync.dma_start(out=out[row : row + P, :], in_=o_tile[:, :])


@lru_cache(maxsize=4)
def _get_nc(reps=1):
    return _build_kernel(reps)


class Runner:
    """Persistent jitted runner mirroring bass2jax.run_bass_via_pjrt, reusable
    across calls (single jit cache entry) so repeated executions can be timed
    back-to-back without recompilation or host round-trips per call."""

    def __init__(self, nc):
        import jax
        import jax.numpy as jnp
        from jax.sharding import Mesh, PartitionSpec
        from jax.experimental.shard_map import shard_map
        from concourse import bass2jax
        from concourse import mybir as _mybir

        bass2jax.install_neuronx_cc_hook()
        self.jax = jax
        self.nc = nc
        partition_name = (
            nc.partition_id_tensor.name if nc.partition_id_tensor else None
        )
        in_names, out_names, out_avals = [], [], []
        for alloc in nc.m.functions[0].allocations:
            if not isinstance(alloc, _mybir.MemoryLocationSet):
                continue
            name = alloc.memorylocations[0].name
            if alloc.kind == "ExternalInput":
                if name != partition_name:
                    in_names.append(name)
            elif alloc.kind == "ExternalOutput":
                out_names.append(name)
                out_avals.append(
                    jax.core.ShapedArray(
                        tuple(alloc.tensor_shape), _mybir.dt.np(alloc.dtype)
                    )
                )
        self.in_names = list(in_names)
        self.out_names = out_names
        self.out_avals = out_avals
        n_params = len(in_names)
        all_names = in_names + out_names
        if partition_name is not None:
            all_names = all_names + [partition_name]

        def _body(*args):
            operands = list(args)
            if partition_name is not None:
                operands.append(bass2jax.partition_id_tensor())
            return tuple(
                bass2jax._bass_exec_p.bind(
                    *operands,
                    out_avals=tuple(out_avals),
                    in_names=tuple(all_names),
                    out_names=tuple(out_names),
                    lowering_input_output_aliases=(),
                    sim_require_finite=True,
                    sim_require_nnan=True,
                    nc=nc,
                )
            )

        devices = jax.devices()[:N_CORES]
        self.mesh = Mesh(np.asarray(devices), ("core",))
        in_specs = (PartitionSpec("core"),) * (n_params + len(out_names))
        out_specs = (PartitionSpec("core"),) * len(out_names)
        self.fn = jax.jit(
            shard_map(
                _body,
                mesh=self.mesh,
                in_specs=in_specs,
                out_specs=out_specs,
                check_rep=False,
            ),
            keep_unused=True,
        )

    def put_inputs(self, in_maps):
        import jax
        from jax.sharding import NamedSharding, PartitionSpec

        concat = [
            np.concatenate([np.asarray(m[name]) for m in in_maps], axis=0)
            for name in self.in_names
        ]
        for aval in self.out_avals:
            concat.append(
                np.zeros((N_CORES * aval.shape[0], *aval.shape[1:]), aval.dtype)
            )
        sh = NamedSharding(self.mesh, PartitionSpec("core"))
        return [jax.device_put(a, sh) for a in concat]

    def __call__(self, dev_inputs):
        return self.fn(*dev_inputs)

    def run_np(self, in_maps):
        outs = self(self.put_inputs(in_maps))
        return [
            {
                name: np.asarray(outs[i]).reshape(N_CORES, *self.out_avals[i].shape)[c]
                for i, name in enumerate(self.out_names)
            }
            for c in range(N_CORES)
        ]


def _prep_inputs(x: np.ndarray, coefficients: np.ndarray):
    x = np.asarray(x, dtype=np.float32)
    coefficients = np.asarray(coefficients, dtype=np.float32)
    # chunk k = j*4+c is degree j+1, i-chunk c, laid out [i within chunk, o];
    # the d=0 term (T_0 == 1) reduces to a bias row added at copy-out.
    c_perm = np.transpose(coefficients, (2, 0, 1))  # (d, i, o)
    bias = np.ascontiguousarray(c_perm[0].sum(axis=0, dtype=np.float64))
    bias = bias.astype(np.float32).reshape(1, OUT_F)
    c_main = c_perm[1:].reshape(N_KCHUNK * P, OUT_F) * C_SCALE
    c_all = np.ascontiguousarray(c_main).astype(MM_NP)

    in_maps = []
    for core in range(N_CORES):
        shard = x[core * B_LOC : (core + 1) * B_LOC]  # (4096, 512)
        xt = np.ascontiguousarray(shard.T)  # (512, 4096)
        in_maps.append({"xT": xt, "Cw": c_all, "bias": bias})
    return in_maps


@lru_cache(maxsize=4)
def _get_runner(reps=1):
    return Runner(_get_nc(reps))


def run_sharded(x, coefficients):
    """Run the 8-core kernel; returns the full (32768, 512) float32 output."""
    in_maps = _prep_inputs(x, coefficients)
    runner = _get_runner()
    results = runner.run_np(in_maps)
    parts = [np.asarray(results[i]["out"]) for i in range(N_CORES)]
    return np.concatenate(parts, axis=0).astype(np.float32)


def _time_runner(runner, dev_in, iters):
    import time

    outs = runner(dev_in)  # warm up
    outs[0].block_until_ready()
    times = []
    for _ in range(iters):
        t0 = time.perf_counter()
        outs = runner(dev_in)
        outs[0].block_until_ready()
        times.append((time.perf_counter() - t0) * 1e9)
    return times


def bench(x, coefficients, iters=12, rep_a=3, rep_b=83):
    """Estimate per-invocation HW time from the slope between two on-device
    repeat counts (fixed ~66-107ms axon RPC overhead cancels). Interleaved
    rounds + median to reject the bimodal RPC jitter. Returns
    (slope_ns, times_a, times_b)."""
    in_maps = _prep_inputs(x, coefficients)
    ra, rb = _get_runner(rep_a), _get_runner(rep_b)
    dev_a = ra.put_inputs(in_maps)
    dev_b = rb.put_inputs(in_maps)
    ta, tb = [], []
    for _ in range(3):
        ta += _time_runner(ra, dev_a, iters // 3 + 1)
        tb += _time_runner(rb, dev_b, iters // 3 + 1)
    med = lambda t: sorted(t)[len(t) // 2]
    slope = (med(tb) - med(ta)) / (rep_b - rep_a)
    return slope, ta, tb


def kernel(x, coefficients):
    return run_sharded(x, coefficients)
